# revision 39
# baseline (speedup 1.0000x reference)
"""Trainium2 Bass kernel for nn_AggAtt (DCNv2-style deformable conv block).

Math (simplified from the reference):
  - om = conv3x3(inp, w_om) + b_om  -> off = om[:2], masks m0..m4 = sigmoid(om[2:7])
  - Only 5 of 9 DCN taps have nonzero mask. Base offset cancels with the conv
    grid, so tap sample positions are simply:
      tap0: (y - h/2, x - w/2) m0 | tap2: (y - h/2, x + w/2) m1
      tap4: (y + off0, x + off1) m2 | tap6: (y + h/2, x - w/2) m3
      tap8: (y,       x + w/2) m4
  - feat[o] = sum_{c,tap} W[o,c,tap] * bilinear(inp[c], pos_tap) * m_tap
  - out = conv1x1(relu(feat + bias)) + b_1x1

Strategy: 8 cores, each handles (batch b, 32 output rows) with an 8-row halo.
Per core: build a channels-last bf16 copy of its input slice in DRAM (PE
transposes), compute om (PE), per-pixel coords/weights (DVE, pixel-major
[128x, 32y] layout), one merged dma_gather per 512-px chunk (9 row-elements
of 2 adjacent columns x 256 channels each), fused scalar_tensor_tensor
combine into S (DVE), PE-transpose S to channel-major, the 5-tap einsum on
the PE with PSUM accumulation, ReLU, and the 1x1 conv.
"""

import os
from contextlib import ExitStack

import numpy as np
import ml_dtypes

import concourse.bass as bass
import concourse.mybir as mybir
import concourse.tile as tile
from concourse import bacc
from concourse.bass_utils import run_bass_kernel_spmd

BF16 = mybir.dt.bfloat16
F32 = mybir.dt.float32
I16 = mybir.dt.int16
ALU = mybir.AluOpType
ACTF = mybir.ActivationFunctionType

B, C, H, W = 2, 256, 128, 128
O, F = 256, 2
NCORES = 8
RPC = H * B // NCORES          # 32 output rows per core
HALO = 8
RTOT = RPC + 2 * HALO          # 48 input rows per core slice
NPX = RPC * W                  # 4096 output pixels per core
NSRC = RTOT * W                # 6144 source pixels per core
CHUNK = 512                    # pixels per pipeline chunk (4 rows)
NCHUNK = NPX // CHUNK          # 8
GPC = CHUNK // 128             # 4 row-groups of 128 px per chunk
G = RPC                        # free dim of pixel-major field tiles (32)
# round-to-int magic: 1.5*2^23 keeps x+MAGIC in the 1.0-ulp range of fp32
# for |x| < 2^22, so (x + MAGIC) - MAGIC == round-to-nearest-integer(x)
MAGIC = float(3 * 2 ** 22)

KSTAGE = int(os.environ.get("KSTAGE", "9"))  # bisect: 1=loads 2=+om/coords 3=+gather 4=+combine 5=+einsum 9=full

# merged-gather element list: (tap-slot κi 0..4, row a/b). tap indices into
# masks m0..m4 and W taps [0,2,4,6,8]; pair id: A=x-w/2, B=x+w/2, C=x+off1
KR = [(0, 'a'), (0, 'b'), (1, 'a'), (1, 'b'), (2, 'a'), (2, 'b'),
      (3, 'a'), (3, 'b'), (4, 'a')]
TAP_PAIR = ['A', 'B', 'C', 'A', 'B']   # colpair per tap-slot
NKR = len(KR)                           # 9
NIDX = NKR * CHUNK                      # 4608 idxs per merged gather


def build(nc: bass.Bass):
    # ---- I/O ----
    xin = nc.dram_tensor("xin", [2, 128, RTOT, 128], F32, kind="ExternalInput")
    whin = nc.dram_tensor("whin", [2, RPC, 128], F32, kind="ExternalInput")
    ygl_in = nc.dram_tensor("ygl", [1, G], F32, kind="ExternalInput")
    rb_in = nc.dram_tensor("rbase", [1, 1], F32, kind="ExternalInput")
    womT = nc.dram_tensor("womT", [128, 18, 7], BF16, kind="ExternalInput")
    wmainT = nc.dram_tensor("wmainT", [128, 5, 2, 2, 128], BF16, kind="ExternalInput")
    w1x1T = nc.dram_tensor("w1x1T", [128, 2, 2], BF16, kind="ExternalInput")
    bom_in = nc.dram_tensor("bom", [7, 1], F32, kind="ExternalInput")
    bmain_in = nc.dram_tensor("bmain", [128, 2], F32, kind="ExternalInput")
    b1_in = nc.dram_tensor("b1", [2, 1], F32, kind="ExternalInput")
    out = nc.dram_tensor("out", [2, RPC, 128], F32, kind="ExternalOutput")

    ident_h = nc.inline_tensor(np.eye(128, dtype=ml_dtypes.bfloat16), "ident")
    identf_h = nc.inline_tensor(np.eye(128, dtype=np.float32), "identf")
    xcol_h = nc.inline_tensor(
        np.arange(128, dtype=np.float32).reshape(128, 1), "xcol")
    # permutation matrix: perm[p, j] = 1 iff p == 16*(j%8) + j//8, so a
    # matmul in.T @ perm reorders pixel columns as j = q*8 + r (p = 16r+q)
    _pm = np.zeros((128, 128), np.float32)
    _jj = np.arange(128)
    _pm[16 * (_jj % 8) + _jj // 8, _jj] = 1.0
    perm_h = nc.inline_tensor(_pm, "perm")

    # ---- internal DRAM scratch ----
    clast = nc.dram_tensor("clast", [NSRC, 256], BF16)      # channels-last
    d4 = nc.dram_tensor("d4", [NCHUNK, NKR * 32 * 16], I16)  # idx pattern

    with tile.TileContext(nc) as tc, ExitStack() as ctx:
        P = ctx.enter_context
        singles = P(tc.tile_pool(name="singles", bufs=1))
        ppad = P(tc.tile_pool(name="ppad", bufs=1))
        pstage = P(tc.tile_pool(name="pstage", bufs=2))
        pom = P(tc.tile_pool(name="pom", bufs=2))
        pfields = P(tc.tile_pool(name="pfields", bufs=1))
        pcoord = P(tc.tile_pool(name="pcoord", bufs=1))
        pg = P(tc.tile_pool(name="pg", bufs=2))
        psx = P(tc.tile_pool(name="psx", bufs=2))
        psc = P(tc.tile_pool(name="psc", bufs=2))
        pfeat = P(tc.tile_pool(name="pfeat", bufs=2))
        pout = P(tc.tile_pool(name="pout", bufs=2))
        # PSUM pools
        ps_small = P(tc.tile_pool(name="ps_small", bufs=2, space="PSUM"))
        ps_t = P(tc.tile_pool(name="ps_t", bufs=2, space="PSUM"))
        ps_f = P(tc.tile_pool(name="ps_f", bufs=2, space="PSUM"))
        ps_idx = P(tc.tile_pool(name="ps_idx", bufs=2, space="PSUM"))

        # ---- load constants / weights ----
        ident = singles.tile([128, 128], BF16, name="ident_sb")
        nc.sync.dma_start(ident[:], ident_h[:])
        identf = singles.tile([128, 128], F32, name="identf_sb")
        nc.sync.dma_start(identf[:], identf_h[:])
        xcol = singles.tile([128, 1], F32, name="xcol_sb")
        nc.sync.dma_start(xcol[:], xcol_h[:])
        perm = singles.tile([128, 128], F32, name="perm_sb")
        nc.sync.dma_start(perm[:], perm_h[:])
        wom_sb = singles.tile([128, 18, 7], BF16, name="wom_sb")
        nc.sync.dma_start(wom_sb[:], womT[:])
        wmain_sb = singles.tile([128, 5, 2, 2, 128], BF16, name="wmain_sb")
        nc.sync.dma_start(wmain_sb[:], wmainT[:])
        w1_sb = singles.tile([128, 2, 2], BF16, name="w1_sb")
        nc.sync.dma_start(w1_sb[:], w1x1T[:])
        bom_sb = singles.tile([7, 1], F32, name="bom_sb")
        nc.sync.dma_start(bom_sb[:], bom_in[:])
        bmain_sb = singles.tile([128, 2], F32, name="bmain_sb")
        nc.sync.dma_start(bmain_sb[:], bmain_in[:])
        b1_sb = singles.tile([2, 1], F32, name="b1_sb")
        nc.sync.dma_start(b1_sb[:], b1_in[:])
        rbase = singles.tile([128, 1], F32, name="rbase_sb")
        nc.sync.dma_start(
            rbase[:],
            bass.AP(tensor=rb_in,
                    offset=0, ap=[[0, 128], [1, 1]]))
        ygl = singles.tile([128, G], F32, name="ygl_sb")
        nc.sync.dma_start(
            ygl[:],
            bass.AP(tensor=ygl_in,
                    offset=0, ap=[[0, 128], [1, G]]))
        wh_sb = singles.tile([32, 2, 128], F32, name="wh_sb")
        nc.sync.dma_start(
            wh_sb[:],
            bass.AP(tensor=whin,
                    offset=0, ap=[[128, 32], [32 * 128, 2], [1, 128]]))

        # ---- load input slice, bf16 cast, padded cols (130 wide) ----
        inp_pad = ppad.tile([128, 2, RTOT, 130], BF16, name="inp_pad")
        for ct in range(2):
            nc.vector.memset(inp_pad[:, ct, :, 0:1], 0.0)
            nc.vector.memset(inp_pad[:, ct, :, 129:130], 0.0)
        for ct in range(2):
            raw = ppad.tile([128, RTOT * 128], BF16, name="raw", tag="raw")
            nc.gpsimd.dma_start(out=raw[:], in_=xin[ct, :, :, :])
            rw = raw[:].rearrange("p (r x) -> p r x", x=128)
            nc.vector.tensor_copy(inp_pad[:, ct, 0:24, 1:129], rw[:, 0:24, :])
            nc.vector.tensor_copy(inp_pad[:, ct, 24:RTOT, 1:129],
                                  rw[:, 24:RTOT, :])

        # ---- P2: om conv per chunk -> F_om [128, G, 7] (pixel-major) ----
        F_om = pfields.tile([128, G, 7], F32, name="F_om")
        for ch in range(NCHUNK if KSTAGE >= 2 else 0):
            pso = ps_small.tile([7, 512], F32, name="pso", tag="pso")
            t = 0
            for dy in range(3):
                for dx in range(3):
                    for ct in range(2):
                        rhs = inp_pad[:, ct, ch * 4 + dy + 7:ch * 4 + dy + 11, dx:dx + 128]
                        nc.tensor.matmul(
                            pso[:], wom_sb[:, t, :], rhs,
                            start=(t == 0), stop=(t == 17))
                        t += 1
            om_sb = pom.tile([7, 512], F32, name="om_sb", tag="om_sb")
            nc.scalar.activation(om_sb[:], pso[:], ACTF.Identity,
                                 bias=bom_sb[:])
            for gg in range(4):
                pfo = ps_idx.tile([128, 128], F32, name="pfo", tag="pidx")
                nc.tensor.matmul(
                    pfo[:, 0:7], om_sb[:, gg * 128:(gg + 1) * 128],
                    identf[0:7, 0:7], is_transpose=True,
                    start=True, stop=True)
                nc.vector.tensor_copy(F_om[:, ch * 4 + gg, :], pfo[:, 0:7])
        # masks m0..m4 live in the free dim now -> sigmoid is legal here
        if KSTAGE >= 2:
            nc.scalar.activation(F_om[:, :, 2:7], F_om[:, :, 2:7], ACTF.Sigmoid)

        # ---- P1: channels-last scratch build (PE transpose route) ----
        for r8 in range(RTOT // 8):
            stg = pstage.tile([128, 8, 2, 128], BF16, name="stg", tag="stg")
            for r2 in range(4):
                pt = ps_t.tile([128, 2, 2, 128], BF16, name="pt", tag="pt")
                for j in range(2):
                    row = r8 * 8 + r2 * 2 + j
                    for ct in range(2):
                        nc.tensor.transpose(
                            pt[:, j, ct, :], inp_pad[:, ct, row, 1:129],
                            ident[:])
                nc.scalar.copy(stg[:, r2 * 2:r2 * 2 + 2, :, :], pt[:])
            nc.sync.dma_start(
                bass.AP(tensor=clast,
                        offset=r8 * 8 * 128 * 256,
                        ap=[[256, 128], [128 * 256, 8], [128, 2], [1, 128]]),
                stg[:])

        # ---- P3: per-pixel coordinate/weight fields (pixel-major) ----
        F_wh = pfields.tile([128, G, 2], F32, name="F_wh")
        for c2 in range(2 if KSTAGE >= 2 else 0):
            pfw = ps_idx.tile([128, 128], F32, name="pfw", tag="pidx")
            nc.tensor.matmul(pfw[:, 0:32], wh_sb[:, c2, :],
                             identf[0:32, 0:32], is_transpose=True,
                             start=True, stop=True)
            nc.vector.tensor_copy(F_wh[:, :, c2], pfw[:, 0:32])

        # idx staging: f = chunk*64 + kr*4 + gg (36 of 64 used per chunk)
        Fidx2 = pfields.tile([128, NCHUNK, 64], F32, name="Fidx2")
        nc.vector.memset(Fidx2[:], 0.0)
        idx_sb = pfields.tile([128, NCHUNK, NKR * 32], I16, name="idx_sb")
        if KSTAGE < 2:
            nc.vector.memset(idx_sb[:], 0)
        Fal = pfields.tile([128, G, 2 * NKR], F32, name="Fal")

        def T(name):
            return pcoord.tile([128, G], F32, name=name, tag=name)

        wF = F_wh[:, :, 0]
        hF = F_wh[:, :, 1]
        coords_on = KSTAGE >= 2

        # funnel rbase through a DVE copy so ops reading it alongside another
        # DMA-loaded tile don't exceed the per-instruction sync-wait limit
        rbase2 = pcoord.tile([128, 1], F32, name="rbase2", tag="rbase2")
        nc.vector.tensor_copy(rbase2[:], rbase[:])
        rbase = rbase2

        def floor_of(src, pfx):
            r = T(pfx + "_r")
            nc.vector.tensor_scalar(r[:], src[:], MAGIC, MAGIC, ALU.add, ALU.subtract)
            g = T(pfx + "_g")
            nc.vector.tensor_tensor(g[:], r[:], src[:], ALU.is_gt)
            f = T(pfx + "_f")
            nc.vector.tensor_tensor(f[:], r[:], g[:], ALU.subtract)
            return f

        def yrows(dy_ap, pfx):
            """rows (a,b): returns (locA, vA, locB, vB)"""
            ys = T(pfx + "_ys")
            nc.vector.tensor_tensor(ys[:], ygl[:], dy_ap, ALU.add)
            y0 = floor_of(ys, pfx + "_y0")
            fy = T(pfx + "_fy")
            nc.vector.tensor_tensor(fy[:], ys[:], y0[:], ALU.subtract)
            w0 = T(pfx + "_w0")
            nc.vector.tensor_scalar(w0[:], fy[:], -1.0, 1.0, ALU.mult, ALU.add)
            res = []
            for nm, base, wgt in (("a", y0, w0), ("b", None, fy)):
                yb = base
                if yb is None:
                    yb = T(pfx + "_y1")
                    nc.vector.tensor_scalar(yb[:], y0[:], 1.0, None, ALU.add)
                rc = T(pfx + "_rc" + nm)
                nc.vector.tensor_scalar(rc[:], yb[:], 0.0, 127.0, ALU.max, ALU.min)
                eq = T(pfx + "_eq" + nm)
                nc.vector.tensor_tensor(eq[:], rc[:], yb[:], ALU.is_equal)
                v = T(pfx + "_v" + nm)
                nc.vector.tensor_tensor(v[:], wgt[:], eq[:], ALU.mult)
                loc = T(pfx + "_loc" + nm)
                nc.vector.tensor_scalar(loc[:], rc[:], rbase[:], float(HALO),
                                        ALU.subtract, ALU.add)
                res += [loc, v]
            return res

        def xpair(dx_ap, pfx):
            """colpair: returns (xi, u0, u1)"""
            xs = T(pfx + "_xs")
            nc.vector.tensor_scalar(xs[:], dx_ap, xcol[:], None, ALU.add)
            x0 = floor_of(xs, pfx + "_x0")
            fx = T(pfx + "_fx")
            nc.vector.tensor_tensor(fx[:], xs[:], x0[:], ALU.subtract)
            w0 = T(pfx + "_w0")
            nc.vector.tensor_scalar(w0[:], fx[:], -1.0, 1.0, ALU.mult, ALU.add)
            xi = T(pfx + "_xi")
            nc.vector.tensor_scalar(xi[:], x0[:], 0.0, 126.0, ALU.max, ALU.min)
            mid = T(pfx + "_mid")
            nc.vector.tensor_tensor(mid[:], xi[:], x0[:], ALU.is_equal)
            em1 = T(pfx + "_em1")
            nc.vector.tensor_scalar(em1[:], x0[:], -1.0, None, ALU.is_equal)
            e127 = T(pfx + "_e127")
            nc.vector.tensor_scalar(e127[:], x0[:], 127.0, None, ALU.is_equal)
            t1 = T(pfx + "_t1")
            nc.vector.tensor_tensor(t1[:], w0[:], mid[:], ALU.mult)
            t2 = T(pfx + "_t2")
            nc.vector.tensor_tensor(t2[:], fx[:], em1[:], ALU.mult)
            u0 = T(pfx + "_u0")
            nc.vector.tensor_tensor(u0[:], t1[:], t2[:], ALU.add)
            t3 = T(pfx + "_t3")
            nc.vector.tensor_tensor(t3[:], fx[:], mid[:], ALU.mult)
            t4 = T(pfx + "_t4")
            nc.vector.tensor_tensor(t4[:], w0[:], e127[:], ALU.mult)
            u1 = T(pfx + "_u1")
            nc.vector.tensor_tensor(u1[:], t3[:], t4[:], ALU.add)
            return xi, u0, u1

        nh2 = T("nh2"); nc.vector.tensor_scalar(nh2[:], hF, -0.5, None, ALU.mult)
        ph2 = T("ph2"); nc.vector.tensor_scalar(ph2[:], hF, 0.5, None, ALU.mult)
        nw2 = T("nw2"); nc.vector.tensor_scalar(nw2[:], wF, -0.5, None, ALU.mult)
        pw2 = T("pw2"); nc.vector.tensor_scalar(pw2[:], wF, 0.5, None, ALU.mult)

        rowsT = yrows(nh2[:], "yT")          # taps 0,2 (top)
        rowsB = yrows(ph2[:], "yB")          # tap 6 (bottom)
        rows4 = yrows(F_om[:, :, 0], "y4")   # tap 4
        loc8 = T("loc8")
        nc.vector.tensor_scalar(loc8[:], ygl[:], rbase[:], float(HALO),
                                ALU.subtract, ALU.add)
        pairA = xpair(nw2[:], "xA")
        pairB = xpair(pw2[:], "xB")
        pairC = xpair(F_om[:, :, 1], "xC")
        pairs = {'A': pairA, 'B': pairB, 'C': pairC}
        taprows = [rowsT, rowsT, rows4, rowsB, [loc8, None, None, None]]

        # idx fields + alpha fields
        for i, (ki, rab) in enumerate(KR):
            locA, vA, locB, vB = taprows[ki]
            loc = locA if rab == 'a' else locB
            v = vA if rab == 'a' else vB
            xi, u0, u1 = pairs[TAP_PAIR[ki]]
            nc.vector.scalar_tensor_tensor(
                Fidx2[:, :, 4 * i:4 * i + 4],
                loc[:].rearrange("p (c g) -> p c g", g=4), 128.0,
                xi[:].rearrange("p (c g) -> p c g", g=4), ALU.mult, ALU.add)
            m_ap = F_om[:, :, 2 + ki]
            if v is None:  # tap8: v == 1
                mv = m_ap
            else:
                mvt = T(f"mv{i}")
                nc.vector.tensor_tensor(mvt[:], m_ap, v[:], ALU.mult)
                mv = mvt[:]
            nc.vector.tensor_tensor(Fal[:, :, 2 * i], mv, u0[:], ALU.mult)
            nc.vector.tensor_tensor(Fal[:, :, 2 * i + 1], mv, u1[:], ALU.mult)

        # PE permutation-matmul moves the pixel dim into the free dim, so the
        # (q=p%16, r=p//16) interleave of the gather idx layout becomes
        # DMA-expressible with <=3-dim APs.
        TpI = pfields.tile([128, 4, 128], I16, name="TpI")
        for fb in range(4):
            pidx = ps_idx.tile([128, 128], F32, name="pidx", tag="pidx")
            nc.tensor.matmul(pidx[:], Fidx2[:, 2 * fb:2 * fb + 2, :], perm[:],
                             start=True, stop=True)
            nc.vector.tensor_copy(TpI[:, fb, :], pidx[:])
        for ch in range(NCHUNK):
            src = TpI[64 * (ch % 2):64 * (ch % 2) + 36, ch // 2, :]
            nc.sync.dma_start(
                bass.AP(tensor=d4, offset=ch * 4608,
                        ap=[[8, 36], [288, 16], [1, 8]]),
                src.rearrange("p (q r) -> p q r", r=8))
        idx_sb = pfields.tile([128, NCHUNK, NKR * 32], I16, name="idx_sb")
        for rep in range(8):
            nc.sync.dma_start(
                idx_sb[16 * rep:16 * rep + 16, :, :],
                bass.AP(tensor=d4, offset=0,
                        ap=[[288, 16], [4608, NCHUNK], [1, 288]]))

        # ---- P4-P6 main per-chunk pipeline ----
        cl_h = clast
        gather_src = bass.AP(tensor=cl_h, offset=0,
                             ap=[[256, NSRC - 1], [1, 512]])
        for ch in range(NCHUNK):
            gta = pg.tile([128, 20, 512], BF16, name="gta", tag="gta")
            gtb = pg.tile([128, 16, 512], BF16, name="gtb", tag="gtb")
            if KSTAGE >= 3:
                nc.gpsimd.dma_gather(
                    gta[:], gather_src, idx_sb[:, ch, 0:160], 2560, 2560, 512,
                    elem_step=256, single_packet=False)
                nc.gpsimd.dma_gather(
                    gtb[:], gather_src, idx_sb[:, ch, 160:288], 2048, 2048, 512,
                    elem_step=256, single_packet=False)
            else:
                nc.vector.memset(gta[:], 0.0)
                nc.vector.memset(gtb[:], 0.0)
            # combine into S (pixel-major)
            spx = psx.tile([128, 4, 5, 256], BF16, name="spx", tag="spx")
            if KSTAGE < 4:
                nc.vector.memset(spx[:], 0.0)
            done = set()
            for i, (ki, rab) in enumerate(KR if KSTAGE >= 4 else []):
                for gg in range(4):
                    for s in range(2):
                        if i < 5:
                            gsl = gta[:, i * 4 + gg, s * 256:(s + 1) * 256]
                        else:
                            gsl = gtb[:, (i - 5) * 4 + gg, s * 256:(s + 1) * 256]
                        asl = Fal[:, ch * 4 + gg, 2 * i + s:2 * i + s + 1]
                        ssl = spx[:, gg, ki, :]
                        if (ki, gg) not in done:
                            if ki < 5:
                                nc.scalar.activation(ssl, gsl, ACTF.Copy,
                                                     scale=asl)
                            else:
                                nc.vector.tensor_scalar(
                                    ssl, gsl, asl, None, ALU.mult)
                            done.add((ki, gg))
                        else:
                            nc.vector.scalar_tensor_tensor(
                                ssl, gsl, asl, ssl, ALU.mult, ALU.add)
            # transpose S to channel-major
            sc = psc.tile([128, 5, 2, 512], BF16, name="sc", tag="sc")
            for ki in range(5 if KSTAGE >= 5 else 0):
                for ct in range(2):
                    pt2 = ps_t.tile([128, 4, 128], BF16, name="pt2", tag="pt")
                    for j in range(4):
                        nc.tensor.transpose(
                            pt2[:, j, :],
                            spx[:, j, ki, ct * 128:(ct + 1) * 128],
                            ident[:])
                    nc.scalar.copy(sc[:, ki, ct, :], pt2[:])
            # einsum + relu
            feat = pfeat.tile([128, 2, 512], BF16, name="feat", tag="feat")
            if KSTAGE < 5:
                nc.vector.memset(feat[:], 0.0)
            for ot in range(2 if KSTAGE >= 5 else 0):
                psf = ps_f.tile([128, 512], F32, name="psf", tag="psf")
                n = 0
                for ki in range(5):
                    for ct in range(2):
                        nc.tensor.matmul(
                            psf[:], wmain_sb[:, ki, ct, ot, :],
                            sc[:, ki, ct, :],
                            start=(n == 0), stop=(n == 9))
                        n += 1
                nc.scalar.activation(feat[:, ot, :], psf[:], ACTF.Relu,
                                     bias=bmain_sb[:, ot:ot + 1])
            # 1x1 conv
            pso1 = ps_small.tile([2, 512], F32, name="pso1", tag="pso")
            for ot in range(2):
                nc.tensor.matmul(pso1[:], w1_sb[:, ot, :], feat[:, ot, :],
                                 start=(ot == 0), stop=(ot == 1))
            osb = pout.tile([2, 512], F32, name="osb", tag="osb")
            nc.scalar.activation(osb[:], pso1[:], ACTF.Identity, bias=b1_sb[:])
            nc.sync.dma_start(
                bass.AP(tensor=out,
                        offset=ch * 512, ap=[[RPC * 128, 2], [1, 512]]),
                osb[:])
    return nc


_bf = ml_dtypes.bfloat16


def _prep_shared(w_om, b_om, weight, bias, w_1x1, b_1x1):
    # womT [c', t=(dy,dx,ct), ch] = w_om[ch, ct*128+c', dy, dx]
    womT = np.ascontiguousarray(
        w_om.reshape(7, 2, 128, 3, 3).transpose(2, 3, 4, 1, 0)
        .reshape(128, 18, 7)).astype(_bf)
    wr = weight.reshape(O, C, 9)[:, :, [0, 2, 4, 6, 8]]  # [O, C, 5]
    # wmainT [c', ki, ct, ot, o'] = wr[ot*128+o', ct*128+c', ki]
    wmainT = np.ascontiguousarray(
        wr.reshape(2, 128, 2, 128, 5).transpose(3, 4, 2, 0, 1)).astype(_bf)
    w1 = w_1x1[:, :, 0, 0]  # [F, O]
    # w1T [o', ot, f] = w1[f, ot*128+o']
    w1T = np.ascontiguousarray(
        w1.reshape(2, 2, 128).transpose(2, 1, 0)).astype(_bf)
    return {
        "womT": womT,
        "wmainT": wmainT,
        "w1x1T": w1T,
        "bom": b_om.reshape(7, 1).astype(np.float32),
        "bmain": np.ascontiguousarray(
            bias.reshape(2, 128).T).astype(np.float32),
        "b1": b_1x1.reshape(2, 1).astype(np.float32),
    }


def kernel(inp, wh, w_om, b_om, weight, bias, w_1x1, b_1x1):
    inp = np.asarray(inp, np.float32)
    wh = np.asarray(wh, np.float32)
    shared = _prep_shared(np.asarray(w_om, np.float32),
                          np.asarray(b_om, np.float32),
                          np.asarray(weight, np.float32),
                          np.asarray(bias, np.float32),
                          np.asarray(w_1x1, np.float32),
                          np.asarray(b_1x1, np.float32))
    in_maps = []
    for core in range(NCORES):
        b = core // (NCORES // B)
        r0 = (core % (NCORES // B)) * RPC
        lo, hi = r0 - HALO, r0 + RPC + HALO
        plo, phi = max(0, -lo), max(0, hi - H)
        sl = inp[b, :, max(0, lo):min(H, hi)]
        sl = np.pad(sl, ((0, 0), (plo, phi), (0, 0)))
        m = dict(shared)
        m["xin"] = np.ascontiguousarray(sl.reshape(2, 128, RTOT, 128))
        m["whin"] = np.ascontiguousarray(wh[b, :, r0:r0 + RPC])
        m["ygl"] = (r0 + np.arange(G, dtype=np.float32)).reshape(1, G)
        m["rbase"] = np.array([[r0]], np.float32)
        in_maps.append(m)

    nc = bacc.Bacc("TRN2")
    build(nc)
    nc.finalize()  # Bacc.compile(): legalizes sync waits (1 per instruction)
    res = run_bass_kernel_spmd(nc, in_maps, list(range(NCORES)),
                               trace=bool(int(os.environ.get("KTRACE", "0"))))
    out = np.empty((B, F, H, W), np.float32)
    for core in range(NCORES):
        b = core // (NCORES // B)
        r0 = (core % (NCORES // B)) * RPC
        out[b, :, r0:r0 + RPC] = res.results[core]["out"]
    kernel.last_results = res
    return out
